# revision 73
# baseline (speedup 1.0000x reference)
"""Distributed Trainium2 kernel for nn_AttentionBsl (LN -> QKV -> 16-head
attention -> output projection) on 8 NeuronCores.

Sharding: token-parallel over the sequence axis, both batches per core.
Core j handles tokens n in [256j, 256j+256) of BOTH batches; its local
512 columns are laid out [b0 tokens | b1 tokens].  Because every core
needs keys/values from every rank (each rank holds both batches' token
slices), K/V are exchanged with three pipelined 8-rank AllGathers
(A = {K head-pairs 0-1, all V}, then K pairs 2-3, then pairs 4-7) whose
outputs are read at the same offsets by every core (SPMD-uniform);
later gathers overlap the attention over earlier head pairs.

LayerNorm is folded into the QKV weights host-side:
  qkv_t = W' x_t * r_t + (-m_t r_t) wg + wb
with W' = W diag(gamma), wg = W' @ 1, wb = W @ beta.  The device scales
x by rstd (per-column broadcast via a K=1 matmul) and appends a 2-row
rank-1 correction to each projection's contraction.

All matmul moving operands are bf16 (full PE feed rate).  attn@V keeps
V stationary ([keys,65] incl. a ones column for softmax sums) and
streams exp(scores) -- the output lands directly in [inner, token]
orientation, eliminating all transposes.  Softmax sums are broadcast
across partitions with K=1 matmuls and inverted in PSUM with the fast
DVE approximate reciprocal.
"""

import sys

if "/opt/trn_rl_repo" not in sys.path:
    sys.path.insert(0, "/opt/trn_rl_repo")

import numpy as np

DIM = 1024
SEQ = 2048
BATCH = 2
HEADS = 16
DH = 64
NCORES = 8
NT = 512           # local tokens per core (both batches)
NB = 256           # local tokens per batch
P = 128
CT = DIM // P      # 8 contraction tiles
NPAIR = HEADS // 2
KTB = SEQ // P     # 16 key tiles per batch
EPS = 1e-5

_CACHE = {}

import os
_STAGE = int(os.environ.get("K_STAGE", "5"))
_SUB = os.environ.get("K_SUB", "e")


def _build(debug=False):
    import concourse.bass as bass  # noqa: F401
    import concourse.mybir as mybir
    import concourse.tile as tile
    from concourse import bacc

    f32 = mybir.dt.float32
    bf16 = mybir.dt.bfloat16
    AF = mybir.ActivationFunctionType
    ALU = mybir.AluOpType

    nc = bacc.Bacc("TRN2", target_bir_lowering=False, debug=False,
                   num_devices=NCORES)

    x_in = nc.dram_tensor("x", [DIM, NT], f32, kind="ExternalInput")
    # bf16 weights packed into f32 words ([in, out] layout, pairs on out)
    wq_in = nc.dram_tensor("wq", [DIM, DIM // 2], f32, kind="ExternalInput")
    wk_in = nc.dram_tensor("wk", [DIM, DIM // 2], f32, kind="ExternalInput")
    wv_in = nc.dram_tensor("wv", [DIM, DIM // 2], f32, kind="ExternalInput")
    wo_in = nc.dram_tensor("wo", [DIM, DIM // 2], f32, kind="ExternalInput")
    # rank-1 LN corrections: rows (wg,wb) x (q,k,v), bf16-packed
    wgb_in = nc.dram_tensor("wgb", [6, DIM // 2], f32, kind="ExternalInput")
    out_ext = nc.dram_tensor("out", [DIM, NT], f32, kind="ExternalOutput")
    if debug:
        dbg_xq = nc.dram_tensor("dbg_xq", [DIM, NT // 2], f32,
                                kind="ExternalOutput")
        dbg_q = nc.dram_tensor("dbg_q", [DIM, NT // 2], f32,
                               kind="ExternalOutput")
        dbg_kg = nc.dram_tensor("dbg_kg", [P, SEQ // 2], f32,
                                kind="ExternalOutput")
        dbg_ao = nc.dram_tensor("dbg_ao", [DIM, NT // 2], f32,
                                kind="ExternalOutput")
        dbg_sm = nc.dram_tensor("dbg_sm", [HEADS, NT], f32,
                                kind="ExternalOutput")

    # AllGather bounce buffers (f32 words, bf16-packed contents).
    # K region: tile ci at rows [(ci//2)*128, +128), cols [(ci%2)*256, +256)
    #           ([128, 512]bf16 packed into [128, 256]f32)
    # V region: V'^T tile tt (local-deint tokens [tt*128,+128)) at rows
    #           [tt*128, +128), cols [0:512]
    cc_in_k0 = nc.dram_tensor("cc_in_k0", [P, NT], f32)
    cc_out_k0 = nc.dram_tensor("cc_out_k0", [NCORES * P, NT], f32,
                               addr_space="Shared")
    cc_in_vv = nc.dram_tensor("cc_in_vv", [4 * P, NT], f32)
    cc_out_vv = nc.dram_tensor("cc_out_vv", [NCORES * 4 * P, NT], f32,
                               addr_space="Shared")
    cc_in_k1 = nc.dram_tensor("cc_in_k1", [P, NT], f32)
    cc_out_k1 = nc.dram_tensor("cc_out_k1", [NCORES * P, NT], f32,
                               addr_space="Shared")
    cc_in_b = nc.dram_tensor("cc_in_b", [2 * P, NT], f32)
    cc_out_b = nc.dram_tensor("cc_out_b", [NCORES * 2 * P, NT], f32,
                              addr_space="Shared")

    with tile.TileContext(nc) as tc:
        with (
            tc.tile_pool(name="const", bufs=1) as constp,
            tc.tile_pool(name="wgbp", bufs=4) as wgbp,
            tc.tile_pool(name="qp", bufs=CT) as qpool,
            tc.tile_pool(name="aop", bufs=CT) as aopool,
            tc.tile_pool(name="wop", bufs=CT) as wop,
        ):
            ones_col = constp.tile([P, 1], f32)
            nc.vector.memset(ones_col[:], 1.0)
            ones_col_bf = constp.tile([P, 1], bf16)
            nc.vector.memset(ones_col_bf[:], 1.0)
            ones_row = constp.tile([1, P], f32)
            nc.vector.memset(ones_row[:], 1.0)
            ones_mat = constp.tile([P, DH], f32)
            nc.vector.memset(ones_mat[:], 1.0)
            # warm the Ln ACT table set during the x DMA (the Exp set loads
            # right after the stats Ln and then stays resident)
            actwarm = constp.tile([1, 2], f32)
            nc.vector.memset(actwarm[:], 1.0)
            nc.scalar.activation(actwarm[:, 0:1], actwarm[:, 0:1], AF.Ln)

            wgb_t = []
            for t in range(3):
                g = wgbp.tile([2, DIM], bf16, name="wgb_t")
                nc.sync.dma_start(out=g[:].bitcast(f32),
                                  in_=wgb_in[2 * t:2 * t + 2, :])
                wgb_t.append(g)

            # =========================================================
            # Phase A: LN stats + folded projections + AllGather.
            # =========================================================
            with (
                tc.tile_pool(name="xp", bufs=CT) as xp,
                tc.tile_pool(name="x2p", bufs=3) as x2p,
                tc.tile_pool(name="wp", bufs=16) as wpool,
                tc.tile_pool(name="xqp", bufs=CT) as xqp,
                tc.tile_pool(name="stats", bufs=10) as statp,
                tc.tile_pool(name="stage", bufs=4) as stagep,
            ):
                x_t = []
                for ci in range(CT):
                    t = xp.tile([P, NT], f32, tag="x", name="x_t")
                    eng = nc.gpsimd if ci % 2 == 0 else nc.sync
                    eng.dma_start(out=t[:],
                                  in_=x_in[ci * P:(ci + 1) * P, :])
                    x_t.append(t)
                wk_t = []
                for ci in range(CT):
                    t = wpool.tile([P, DIM], bf16, tag="w", name="wk_t")
                    nc.gpsimd.dma_start(out=t[:].bitcast(f32),
                                        in_=wk_in[ci * P:(ci + 1) * P, :])
                    wk_t.append(t)
                # raw bf16 copy of x: lets the K projection run before the
                # LN stats are known (K = rr o (W'x_bf) + C applied after).
                xb_t = []
                for ci in range(CT):
                    xb = x2p.tile([P, NT], bf16, tag="xb", name="xb_t",
                                  bufs=CT)
                    nc.vector.tensor_copy(xb[:], x_t[ci][:])
                    xb_t.append(xb)

                stats = {}

                def emit_stats(lnps):
                    ps_s = lnps.tile([1, NT], f32, tag="ln", bufs=2)
                    ps_q = lnps.tile([1, NT], f32, tag="ln", bufs=2)
                    for ci in range(CT):
                        x2 = x2p.tile([P, NT], bf16, tag="x2", name="x2_t",
                                      bufs=3)
                        nc.vector.tensor_tensor(x2[:], x_t[ci][:], x_t[ci][:],
                                                ALU.mult)
                        nc.tensor.matmul(ps_s[:], ones_col[:], x_t[ci][:],
                                         start=(ci == 0), stop=(ci == CT - 1))
                        nc.tensor.matmul(ps_q[:], ones_col_bf[:], x2[:],
                                         start=(ci == 0), stop=(ci == CT - 1))
                    mean = statp.tile([1, NT], f32, tag="st", name="mean")
                    nc.vector.tensor_scalar_mul(mean[:], ps_s[:], 1.0 / DIM)
                    var = statp.tile([1, NT], f32, tag="st", name="var")
                    nc.vector.tensor_scalar_mul(var[:], ps_q[:], 1.0 / DIM)
                    m2 = statp.tile([1, NT], f32, tag="st", name="m2")
                    nc.vector.tensor_tensor(m2[:], mean[:], mean[:], ALU.mult)
                    nc.vector.tensor_tensor(var[:], var[:], m2[:],
                                            ALU.subtract)
                    # rstd = exp(-0.5 * ln(var + eps))
                    nc.vector.tensor_scalar_add(var[:], var[:], EPS)
                    lnv = statp.tile([1, NT], f32, tag="st", name="lnv")
                    nc.scalar.activation(lnv[:], var[:], AF.Ln)
                    rstd = statp.tile([1, NT], f32, tag="st", name="rstd")
                    nc.scalar.activation(rstd[:], lnv[:], AF.Exp, scale=-0.5)
                    nmr = statp.tile([1, NT], f32, tag="st", name="nmr")
                    nc.vector.tensor_tensor(nmr[:], mean[:], rstd[:],
                                            ALU.mult)
                    nc.vector.tensor_scalar_mul(nmr[:], nmr[:], -1.0)
                    corr2 = statp.tile([2, NT], bf16, tag="c2", name="corr2")
                    nc.vector.memset(corr2[:], 1.0)
                    nc.vector.tensor_copy(corr2[0:1, :], nmr[:])
                    stats["rstd"] = rstd
                    stats["corr2"] = corr2

                with tc.tile_pool(name="pjps", bufs=3, space="PSUM") as pjps:
                    kps = {}
                    kraw = {}

                    def k_chain(ot):
                        ps = pjps.tile([P, NT], f32, tag="pj", name="kps")
                        for ci in range(CT):
                            nc.tensor.matmul(
                                ps[:], wk_t[ci][:, ot * P:(ot + 1) * P],
                                xb_t[ci][:],
                                start=(ci == 0), stop=(ci == CT - 1))
                        kps[ot] = ps

                    def k_evac(ot):
                        kr = statp.tile([P, NT], f32, tag="kraw", bufs=6,
                                        name="kraw")
                        nc.vector.tensor_copy(kr[:], kps[ot][:])
                        kraw[ot] = kr

                    def k_stage(ot, ckpool, src):
                        ck = ckpool.tile([P, NT], f32, tag="ck", bufs=2)
                        nc.tensor.matmul(
                            ck[:], wgb_t[1][:, ot * P:(ot + 1) * P],
                            corr2[:], start=True, stop=True)
                        tmp = stagep.tile([P, NT], f32, tag="ktmp",
                                          name="ktmp", bufs=2)
                        nc.vector.tensor_tensor(tmp[:], src[:], rr_sb[:],
                                                ALU.mult)
                        st = stagep.tile([P, NT], bf16, tag="kstg",
                                         name="kstg", bufs=2)
                        nc.vector.tensor_tensor(st[:], tmp[:], ck[:], ALU.add)
                        blk = ot // 2
                        if blk == 0:
                            kdst = cc_in_k0[:]
                        elif blk == 1:
                            kdst = cc_in_k1[:]
                        else:
                            kdst = cc_in_b[(blk - 2) * P:(blk - 1) * P, :]
                        nc.sync.dma_start(
                            out=kdst[:, (ot % 2) * 256:(ot % 2 + 1) * 256],
                            in_=st[:].bitcast(f32))

                    # ---- K chains 0-3 with LN stats interleaved ----
                    with tc.tile_pool(name="lnps", bufs=3,
                                      space="PSUM") as lnps:
                        for ot in range(4):
                            k_chain(ot)
                            if ot == 1:
                                emit_stats(lnps)
                            if ot == 2:
                                rr = lnps.tile([P, NT], f32, tag="rr", bufs=1)
                                nc.tensor.matmul(rr[:], ones_row[:],
                                                 stats["rstd"][:],
                                                 start=True, stop=True)
                                rr_sb = statp.tile([P, NT], f32, tag="rrs",
                                                   name="rr_sb")
                                nc.vector.tensor_copy(rr_sb[:], rr[:])
                    corr2 = stats["corr2"]
                    rstd = stats["rstd"]

                    # stage K block 0 NOW (gates AG-K0); evacuate K1 raw
                    with tc.tile_pool(name="ckps", bufs=2,
                                      space="PSUM") as ckps:
                        k_stage(0, ckps, kps[0])
                        k_stage(1, ckps, kps[1])
                        k_evac(2)
                        k_evac(3)

                    # x' = x * rstd for the V/Q projections
                    xq_t = []
                    for ci in range(CT):
                        xq = xqp.tile([P, NT], bf16, tag="xq", name="xq_t")
                        nc.vector.tensor_tensor(xq[:], xb_t[ci][:], rr_sb[:],
                                                ALU.mult)
                        xq_t.append(xq)

                    # ---- V'^T projection -> cc_in_a ----
                    wv_t = []
                    for ci in range(CT):
                        t = wpool.tile([P, DIM], bf16, tag="w", name="wv_t")
                        nc.gpsimd.dma_start(out=t[:].bitcast(f32),
                                            in_=wv_in[ci * P:(ci + 1) * P, :])
                        wv_t.append(t)
                    with tc.tile_pool(name="vps", bufs=2,
                                      space="PSUM") as vps:
                        for tt in range(4):
                            ps = vps.tile([P, DIM], f32, tag="vpj",
                                          name="vps_t")
                            for half in range(2):
                                sl = slice(half * 512, (half + 1) * 512)
                                for ci in range(CT):
                                    nc.tensor.matmul(
                                        ps[:, sl],
                                        xq_t[ci][:, tt * P:(tt + 1) * P],
                                        wv_t[ci][:, sl],
                                        start=(ci == 0), stop=False)
                                nc.tensor.matmul(
                                    ps[:, sl],
                                    corr2[:, tt * P:(tt + 1) * P],
                                    wgb_t[2][:, sl],
                                    start=False, stop=True)
                            st = stagep.tile([P, DIM], bf16, tag="stg",
                                             name="vstg")
                            nc.vector.tensor_copy(st[:], ps[:])
                            nc.sync.dma_start(
                                out=cc_in_vv[tt * P:(tt + 1) * P, :],
                                in_=st[:].bitcast(f32))

                    # ---- K chains 4-7, evacuated raw ----
                    for ot in range(4, CT):
                        k_chain(ot)
                        k_evac(ot)

                    # ---- Q projection (overlaps AG-A) ----
                    wq_t = []
                    for ci in range(CT):
                        t = wpool.tile([P, DIM], bf16, tag="w", name="wq_t")
                        nc.sync.dma_start(out=t[:].bitcast(f32),
                                          in_=wq_in[ci * P:(ci + 1) * P, :])
                        wq_t.append(t)
                    q_t = []
                    for ot in range(CT):
                        ps = pjps.tile([P, NT], f32, tag="pj", name="qps")
                        for ci in range(CT):
                            nc.tensor.matmul(
                                ps[:], wq_t[ci][:, ot * P:(ot + 1) * P],
                                xq_t[ci][:],
                                start=(ci == 0), stop=False)
                        nc.tensor.matmul(
                            ps[:], wgb_t[0][:, ot * P:(ot + 1) * P],
                            corr2[:], start=False, stop=True)
                        qt_ = qpool.tile([P, NT], bf16, tag="q", name="q_t")
                        nc.vector.tensor_copy(qt_[:], ps[:])
                        q_t.append(qt_)
                    if debug:
                        for ci in range(CT):
                            nc.sync.dma_start(
                                out=dbg_xq[ci * P:(ci + 1) * P, :],
                                in_=xq_t[ci][:].bitcast(f32))
                            nc.sync.dma_start(
                                out=dbg_q[ci * P:(ci + 1) * P, :],
                                in_=q_t[ci][:].bitcast(f32))
                    if _STAGE == 1:
                        for ci in range(CT):
                            nc.sync.dma_start(
                                out=out_ext[ci * P:(ci + 1) * P, 0:NT // 2],
                                in_=q_t[ci][:].bitcast(f32))

                    # ---- late staging of K1/B (their DVE scale TTs sit
                    # behind everything above on the in-order vector queue,
                    # so AG-A's inputs are unambiguously ready first) ----
                    with tc.tile_pool(name="ckps2", bufs=2,
                                      space="PSUM") as ckps2:
                        for ot in range(2, CT):
                            k_stage(ot, ckps2, kraw[ot])

                    # ---- pipelined AllGathers, in consumption order:
                    # K0 (head pairs 0-1), all V, K1 (pairs 2-3),
                    # B = {K2, K3} (pairs 4-7).
                    if _STAGE >= 2:
                        cc_seq = [(cc_in_k0, cc_out_k0),
                                  (cc_in_vv, cc_out_vv),
                                  (cc_in_k1, cc_out_k1),
                                  (cc_in_b, cc_out_b)]
                        for ci_t, co_t in cc_seq:
                            nc.gpsimd.collective_compute(
                                "AllGather", ALU.bypass,
                                replica_groups=[list(range(NCORES))],
                                ins=[ci_t.ap().opt()],
                                outs=[co_t.ap().opt()],
                            )


            # prefetch w_out during phase B
            wo_t = []
            for ci in range(CT):
                t = wop.tile([P, DIM], bf16, tag="wo", name="wo_t")
                nc.gpsimd.dma_start(out=t[:].bitcast(f32),
                                    in_=wo_in[ci * P:(ci + 1) * P, :])
                wo_t.append(t)

            # =========================================================
            # Phase B: attention.
            # =========================================================
            with (
                tc.tile_pool(name="kgp", bufs=2) as kgp,
                tc.tile_pool(name="vgp", bufs=2 * KTB) as vgp,
                tc.tile_pool(name="attnT", bufs=12) as atp,
                tc.tile_pool(name="sumst", bufs=4) as sumstp,
                tc.tile_pool(name="rbp", bufs=HEADS) as rbp,
            ):
                def load_kg(hp, eng=None):
                    t = kgp.tile([P, 2 * SEQ], bf16, tag="kg", name="kg_t")
                    blk = hp // 2
                    if blk == 0:
                        cc_kv = cc_out_k0.rearrange("(p r) c -> r p c",
                                                    p=NCORES)
                    elif blk == 1:
                        cc_kv = cc_out_k1.rearrange("(p r) c -> r p c",
                                                    p=NCORES)
                    else:
                        cc_kv = cc_out_b.rearrange(
                            "(p r) c -> r p c",
                            p=NCORES)[(blk - 2) * P:(blk - 1) * P]
                    for b in range(BATCH):
                        coff = (hp % 2) * 256 + b * 128
                        src = cc_kv[:, :, coff:coff + 128]
                        (eng or nc.sync).dma_start(
                            out=t[:, b * SEQ:(b + 1) * SEQ].bitcast(
                                f32).rearrange("p (r c) -> p r c", r=NCORES),
                            in_=src)
                    return t

                # hp0's gather rides the scalar engine's idle DMA ring so it
                # isn't queued behind the 32 V-tile loads on sync.
                kg_next = load_kg(0, nc.scalar) if _STAGE >= 2 else None

                # V tiles: v_g[b][kt] = [128 keys, 16 heads, 64+1], loaded in
                # need-order (b, kt) on the sync queue.
                v_g = [[None] * KTB for _ in range(BATCH)]
                for b in range(BATCH if _STAGE >= 2 else 0):
                    for kt in range(KTB):
                        pk, half = kt // 2, kt % 2
                        tt = b * 2 + half
                        t = vgp.tile([P, HEADS, DH + 1], bf16, tag="vg",
                                     name="v_g")
                        nc.vector.memset(t[:, :, DH:DH + 1], 1.0)
                        src = cc_out_vv[pk * 4 * P + tt * P:
                                        pk * 4 * P + (tt + 1) * P, :]
                        nc.sync.dma_start(
                            out=t[:, :, 0:DH],
                            in_=src.bitcast(bf16).rearrange(
                                "p (h d) -> p h d", h=HEADS))
                        v_g[b][kt] = t
                if debug and kg_next is not None:
                    nc.sync.dma_start(out=dbg_kg[:],
                                      in_=kg_next[:, 0:SEQ].bitcast(f32))
                if _STAGE == 2:
                    nc.sync.dma_start(out=out_ext[0:P, :],
                                      in_=kg_next[:, 0:2 * NT].bitcast(f32))
                ao_t = [aopool.tile([P, NT], bf16, tag="ao", name="ao")
                        for _ in range(CT)]
                sums_sb = [None] * HEADS

                with (
                    tc.tile_pool(name="scps", bufs=2, space="PSUM") as scps,
                    tc.tile_pool(name="avps", bufs=4, space="PSUM") as avps,
                ):
                    for hp in range(NPAIR if _STAGE >= 3 else 0):
                        kg = kg_next
                        if hp + 1 < NPAIR:
                            kg_next = load_kg(hp + 1)
                        av = ([avps.tile([DH + 1, NT], f32, tag="av",
                                         name="av") for _ in range(2)]
                              if _SUB >= "c" else None)
                        for b in range(BATCH):
                            # attn@V lags the scores/exp by one group so the
                            # in-order PE queue never waits on the current
                            # group's exp.
                            lag = None
                            for ktp in range(KTB // 2):
                                if _SUB < "a":
                                    continue
                                # col blocks: [h0kt0 | h0kt1 | h1kt0 | h1kt1]
                                # so the row-group-concurrent (h0, h1) pair
                                # writes DIFFERENT psum banks.
                                sc = scps.tile([P, 4 * NB], f32, tag="sc",
                                               name="sc")
                                quads = [(0, 2 * ktp, 0), (2, 2 * ktp, 1),
                                         (1, 2 * ktp + 1, 0),
                                         (3, 2 * ktp + 1, 1)]
                                for qi, kt, hi in quads:
                                    nc.tensor.matmul(
                                        sc[:, qi * NB:(qi + 1) * NB],
                                        kg[hi * DH:(hi + 1) * DH,
                                           b * SEQ + kt * P:
                                           b * SEQ + (kt + 1) * P],
                                        q_t[hp][hi * DH:(hi + 1) * DH,
                                                b * NB:(b + 1) * NB],
                                        start=True, stop=True)
                                if _SUB < "b":
                                    continue
                                at = atp.tile([P, 4 * NB], bf16, tag="at",
                                              name="at")
                                nc.scalar.activation(
                                    at[:], sc[:], AF.Exp,
                                    scale=float(1.0 / np.sqrt(DH)))
                                if _SUB < "c":
                                    continue
                                if lag is not None:
                                    for qi, kt, hi in lag:
                                        nc.tensor.matmul(
                                            av[hi][:, b * NB:(b + 1) * NB],
                                            v_g[b][kt][:, hp * 2 + hi, :],
                                            lag_at[:, qi * NB:(qi + 1) * NB],
                                            start=(kt == 0),
                                            stop=(kt == KTB - 1))
                                lag, lag_at = quads, at
                            if _SUB >= "c" and lag is not None:
                                for qi, kt, hi in lag:
                                    nc.tensor.matmul(
                                        av[hi][:, b * NB:(b + 1) * NB],
                                        v_g[b][kt][:, hp * 2 + hi, :],
                                        lag_at[:, qi * NB:(qi + 1) * NB],
                                        start=(kt == 0), stop=(kt == KTB - 1))
                        # evacuate: rows 0:64 per head -> ao (unnormalized),
                        # row 64 (sums) -> reciprocal -> base-0 row tiles
                        if _SUB < "d":
                            continue
                        nc.vector.tensor_copy(ao_t[hp][0:DH, :],
                                              av[0][0:DH, :])
                        tmp1 = sumstp.tile([DH, NT], bf16, tag="tmp1",
                                           name="tmp1", bufs=2)
                        nc.vector.tensor_copy(tmp1[:], av[1][0:DH, :])
                        nc.sync.dma_start(out=ao_t[hp][DH:P, :],
                                          in_=tmp1[:])
                        if _SUB < "e":
                            continue
                        for hi in range(2):
                            stg = sumstp.tile([DH + 1, NT], f32, tag="sm",
                                              name="sumstg", bufs=HEADS)
                            nc.vector.tensor_copy(stg[DH:DH + 1, :],
                                                  av[hi][DH:DH + 1, :])
                            sums_sb[2 * hp + hi] = stg

                # tail: broadcast the sums across partitions, invert on
                # the psum tile, normalize ao
                with tc.tile_pool(name="bcps", bufs=2, space="PSUM") as bcps:
                    for hp in range(NPAIR if _STAGE >= 4 else 0):
                        bc = bcps.tile([P, NT], f32, tag="bc", name="bc")
                        nc.tensor.matmul(bc[0:DH, :],
                                         ones_mat[DH:DH + 1, 0:DH],
                                         sums_sb[2 * hp][DH:DH + 1, :],
                                         start=True, stop=True)
                        nc.tensor.matmul(bc[DH:P, :],
                                         ones_mat[DH:DH + 1, 0:DH],
                                         sums_sb[2 * hp + 1][DH:DH + 1, :],
                                         start=True, stop=True,
                                         skip_group_check=True)
                        nc.vector.reciprocal_approx_fast(out=bc[:], in_=bc[:])
                        nc.vector.tensor_tensor(ao_t[hp][:], ao_t[hp][:],
                                                bc[:], ALU.mult)

                if _STAGE == 3 and _SUB >= "d":
                    for ci in range(CT):
                        nc.sync.dma_start(
                            out=out_ext[ci * P:(ci + 1) * P, 0:NT // 2],
                            in_=ao_t[ci][:].bitcast(f32))
                if debug:
                    for ci in range(CT):
                        nc.sync.dma_start(out=dbg_ao[ci * P:(ci + 1) * P, :],
                                          in_=ao_t[ci][:].bitcast(f32))
                    for hg in range(HEADS):
                        nc.sync.dma_start(out=dbg_sm[hg:hg + 1, :],
                                          in_=sums_sb[hg][DH:DH + 1, :])

            # =========================================================
            # Phase C: output projection.
            # =========================================================
            with (
                tc.tile_pool(name="outsb", bufs=3) as outp,
                tc.tile_pool(name="ops", bufs=4, space="PSUM") as ops,
            ):
                for ot in range(CT if _STAGE >= 5 else 0):
                    ps = ops.tile([P, NT], f32, tag="o", name="ops_t")
                    for ci in range(CT):
                        nc.tensor.matmul(
                            ps[:], wo_t[ci][:, ot * P:(ot + 1) * P],
                            ao_t[ci][:],
                            start=(ci == 0), stop=(ci == CT - 1))
                    ost = outp.tile([P, NT], f32, tag="ou", name="ost")
                    nc.vector.tensor_copy(ost[:], ps[:])
                    nc.sync.dma_start(out=out_ext[ot * P:(ot + 1) * P, :],
                                      in_=ost[:])

    nc.compile()
    return nc


def _get_nc(debug=False):
    key = ("nc", debug)
    if key not in _CACHE:
        _CACHE[key] = _build(debug)
    return _CACHE[key]


def _bf16_pack(a):
    a = np.ascontiguousarray(a, np.float32)
    try:
        import ml_dtypes
        return a.astype(ml_dtypes.bfloat16).view(np.float32)
    except ImportError:
        # round-to-nearest-even bf16, packed pairwise into f32 words
        u = a.view(np.uint32)
        hi = ((u + 0x7FFF + ((u >> 16) & 1)) >> 16).astype(np.uint32)
        hi = hi.reshape(a.shape[:-1] + (a.shape[-1] // 2, 2))
        packed = hi[..., 0] | (hi[..., 1] << 16)
        return np.ascontiguousarray(packed).view(np.float32)


def kernel(x, w_qkv, w_out, ln_gamma, ln_beta, _profile=False, _debug=False):
    from concourse.bass_utils import run_bass_kernel_spmd

    x = np.asarray(x, np.float32)
    w_qkv = np.asarray(w_qkv, np.float32)
    w_out = np.asarray(w_out, np.float32)
    g = np.asarray(ln_gamma, np.float32)
    bt = np.asarray(ln_beta, np.float32)

    ws = []
    rows = []
    for t in range(3):
        W = w_qkv[t * DIM:(t + 1) * DIM, :]       # [out, in]
        Wp = W * g[None, :]
        ws.append(_bf16_pack(Wp.T))               # [in, out] bf16-packed
        rows.append(Wp.sum(axis=1))               # wg
        rows.append(W @ bt)                       # wb
    wq_p, wk_p, wv_p = ws
    wgb_p = _bf16_pack(np.stack(rows))            # [6, 1024] -> [6, 512]
    wo_p = _bf16_pack(w_out.T)

    in_maps = []
    for j in range(NCORES):
        xj = np.concatenate(
            [x[:, NB * j:NB * (j + 1), 0], x[:, NB * j:NB * (j + 1), 1]],
            axis=1)
        in_maps.append({
            "x": np.ascontiguousarray(xj),
            "wq": wq_p, "wk": wk_p, "wv": wv_p, "wo": wo_p,
            "wgb": wgb_p,
        })

    nc = _get_nc(_debug)
    res = run_bass_kernel_spmd(nc, in_maps, core_ids=list(range(NCORES)),
                               trace=_profile)
    if _profile:
        _CACHE["last_result"] = res

    out = np.empty((DIM, SEQ, BATCH), np.float32)
    for j in range(NCORES):
        r = res.results[j]["out"]
        out[:, NB * j:NB * (j + 1), 0] = r[:, 0:NB]
        out[:, NB * j:NB * (j + 1), 1] = r[:, NB:NT]
    if _debug:
        _CACHE["dbg"] = res.results
    return out
